# revision 43
# baseline (speedup 1.0000x reference)
"""Causal single-head attention + RoPE on 8 TRN2 NeuronCores (Bass/Tile SPMD), v2.

Same sharding/numerics as v1 (core c -> batch c//2, key-parity c%2, host
flash-combine; fp16 hi/lo 3-pass projections, 2-matmul stacked scores).
Restructured for engine balance:
  - rope pair-swap via Pool TT (partition-offset slices) instead of fp32 matmul
  - phase B row-max: tail-first psum layout, fused mask+reduce (TTR), big DMAs
  - phase B/C interleaved per query-block group; m flattened in 4 groups
  - o_out written [65, S] (host transposes); no per-tile PE transposes
"""
import numpy as np

S, E, DH, B, P = 4096, 1024, 64, 4, 128
NEG = -1.0e30
NEG16 = -60000.0
_CACHE = {}


def _build_program():
    import concourse.tile as tile
    import concourse.mybir as mybir
    from concourse import bacc
    from concourse.masks import make_identity
    from contextlib import ExitStack

    dt = mybir.dt
    f32, f16 = dt.float32, dt.float16
    AF = mybir.ActivationFunctionType
    ALU = mybir.AluOpType
    AX = mybir.AxisListType

    nc = bacc.Bacc("TRN2", target_bir_lowering=False, debug=False, num_devices=8)

    xth = nc.dram_tensor("xth", [8, P, S], f16, kind="ExternalInput").ap()
    xtl = nc.dram_tensor("xtl", [8, P, S], dt.float8e5, kind="ExternalInput").ap()
    wpk = nc.dram_tensor("wpk", [P, 8, 320], f16, kind="ExternalInput").ap()
    cosr = nc.dram_tensor("cosr", [P, S], f32, kind="ExternalInput").ap()
    sinw = nc.dram_tensor("sinw", [P, S], f32, kind="ExternalInput").ap()
    wpk8 = nc.dram_tensor("wpk8", [P, 8, 128], dt.float8e5, kind="ExternalInput").ap()
    stairs = nc.dram_tensor("stairs", [P, P], f16, kind="ExternalInput").ap()
    mrhs = nc.dram_tensor("mrhs", [1, 512], f16, kind="ExternalInput").ap()
    stm = nc.dram_tensor("stm", [P, 1024], f16, kind="ExternalInput").ap()
    o_out = nc.dram_tensor("o_out", [65, S], f32, kind="ExternalOutput").ap()
    m_out = nc.dram_tensor("m_out", [32, P], f32, kind="ExternalOutput").ap()

    def cu_of(p):
        return min((p + 3) // 2, 16)

    with tile.TileContext(nc) as tc, ExitStack() as ctx:
        const = ctx.enter_context(tc.tile_pool(name="const", bufs=1))
        xpool = ctx.enter_context(tc.tile_pool(name="xpool", bufs=2))
        work = ctx.enter_context(tc.tile_pool(name="work", bufs=4))
        apool = ctx.enter_context(tc.tile_pool(name="apool", bufs=10))
        psA = ctx.enter_context(tc.tile_pool(name="psA", bufs=3, space="PSUM"))
        psB = ctx.enter_context(tc.tile_pool(name="psB", bufs=2, space="PSUM"))
        psO = ctx.enter_context(tc.tile_pool(name="psO", bufs=1, space="PSUM"))
        dram = ctx.enter_context(tc.tile_pool(name="dram", bufs=1, space="DRAM"))

        # --- DMA issue order tuned: weights, x0, x1, trig0, x2, trig1, x3 ---
        w_sb = const.tile([P, 8, 320], f16)
        nc.scalar.dma_start(w_sb[:], wpk[:])

        def load_x_chunk(sh):
            xh_sb = xpool.tile([P, 8, 1024], f16, tag="xh")
            xl_sb = xpool.tile([P, 8, 1024], dt.float8e5, tag="xl")
            c0 = slice(sh * 1024, sh * 1024 + 512)
            c1 = slice(sh * 1024 + 512, (sh + 1) * 1024)
            nc.sync.dma_start(xh_sb[:, :, 0:512],
                              xth[:, :, c0].rearrange("e p w -> p e w"))
            nc.sync.dma_start(xh_sb[:, :, 512:1024],
                              xth[:, :, c1].rearrange("e p w -> p e w"))
            nc.sync.dma_start(xl_sb[:], xtl[:, :, sh * 1024:(sh + 1) * 1024]
                              .rearrange("e p w -> p e w"))
            return xh_sb, xl_sb

        cos_sb = const.tile([P, S], f32)
        sin_sb = const.tile([P, S], f32)
        nc.scalar.dma_start(cos_sb[:, 0:2048], cosr[:, 0:2048])
        nc.scalar.dma_start(sin_sb[:, 0:2048], sinw[:, 0:2048])
        xs = [load_x_chunk(0), load_x_chunk(1)]
        nc.scalar.dma_start(cos_sb[:, 2048:4096], cosr[:, 2048:4096])
        nc.scalar.dma_start(sin_sb[:, 2048:4096], sinw[:, 2048:4096])
        xs.append(load_x_chunk(2))
        xs.append(load_x_chunk(3))
        w8_sb = const.tile([P, 8, 128], dt.float8e5)
        nc.scalar.dma_start(w8_sb[:], wpk8[:])
        stairs_sb = const.tile([P, P], f16)
        nc.scalar.dma_start(stairs_sb[:], stairs[:])
        mrhs_sb = const.tile([1, 512], f16)
        nc.scalar.dma_start(mrhs_sb[:], mrhs[:])
        nr1 = const.tile([1, P], f16)
        nc.vector.memset(nr1[:], NEG16)
        stm_sb = const.tile([P, 1024], f16)
        nc.scalar.dma_start(stm_sb[:], stm[:])
        id16 = const.tile([P, P], f16)
        make_identity(nc, id16[:])
        id32 = const.tile([P, P], f32)
        make_identity(nc, id32[:])

        QKh = const.tile([P, S], f16)        # rows 0:64 Qh, 64:128 Kh (full seq)
        LoQK = const.tile([P, S], f16)       # rows 0:64 Ql, 64:128 Kl
        QrHH = const.tile([P, S], f16)       # [Qh;Qh]
        QrLM = const.tile([65, S], f16)      # [Ql;m]
        KrHL = const.tile([P, 2048], f16)    # [Kh;Kl] stripe
        KrH1 = const.tile([65, 2048], f16)   # [Kh;-1]
        Vaug = const.tile([P, 16, 65], f16)
        m_sb = const.tile([P, 32], f32)
        nc.vector.memset(Vaug[:, :, 64:65], 1.0)
        nc.vector.memset(KrH1[64:65, :], -1.0)

        # ---------- phase A: projections + rope (emitted per chunk below) ----
        def phase_a(sh):
            xh_sb, xl_sb = xs[sh]

            # V projection directly in [pos, d]: stationary = x block (stripe
            # = rolled-even 128-blocks, sub-index 0 of each pair)
            for u in range(4):
                j = 4 * sh + u
                bc = slice(256 * u, 256 * u + 128)
                vps = psA.tile([P, 512], f32, tag="pk")
                for ec in range(8):
                    nc.tensor.matmul(vps[:, 0:64], xh_sb[:, ec, bc],
                                     w_sb[:, ec, 128:192],
                                     start=(ec == 0), stop=(ec == 7))
                nc.vector.tensor_copy(Vaug[:, j, 0:64], vps[:, 0:64])

            for jj in range(2):
                J = sh * 2 + jj
                cols = slice(J * 512, (J + 1) * 512)
                pk = psA.tile([P, 512], f32, tag="pk")
                for ec in range(8):
                    rhs_h = xh_sb[:, ec, jj * 512:(jj + 1) * 512]
                    nc.tensor.matmul(pk[:], w_sb[:, ec, 0:128], rhs_h,
                                     start=(ec == 0), stop=False)
                    nc.tensor.matmul(pk[:], w_sb[:, ec, 192:320], rhs_h,
                                     start=False, stop=False)
                for e2 in range(4):
                    rhs_l = xl_sb[:, 2 * e2:2 * e2 + 2, jj * 512:(jj + 1) * 512]
                    nc.tensor.matmul(pk[:], w8_sb[:, 2 * e2:2 * e2 + 2, :], rhs_l,
                                     start=False, stop=(e2 == 3),
                                     perf_mode=mybir.MatmulPerfMode.DoubleRow)
                u = work.tile([P, 512], f32, tag="u")
                nc.vector.tensor_tensor(u[:], pk[:], sin_sb[:, cols], ALU.mult)
                r = work.tile([P, 512], f32, tag="r")
                nc.vector.tensor_tensor(r[:], pk[:], cos_sb[:, cols], ALU.mult)
                # us = swap32(u) via Pool copies (cross-partition TT is illegal)
                us = work.tile([P, 512], f32, tag="us")
                for g in range(4):
                    d = slice(g * 32, (g + 1) * 32)
                    sw = slice((g * 32) ^ 32, ((g * 32) ^ 32) + 32)
                    nc.gpsimd.tensor_copy(us[d, :], u[sw, :])
                nc.gpsimd.tensor_tensor(r[:], r[:], us[:], ALU.add)
                nc.vector.tensor_copy(QKh[:, cols], r[:])
                nc.gpsimd.tensor_tensor(LoQK[:, cols], r[:], QKh[:, cols],
                                        ALU.subtract)
                nc.vector.tensor_copy(QrHH[0:64, cols], QKh[0:64, cols])
                nc.gpsimd.tensor_copy(QrHH[64:128, cols], QKh[0:64, cols])
                nc.vector.tensor_copy(QrLM[0:64, cols], LoQK[0:64, cols])

            # stripe gathers for this chunk (rolled-even blocks)
            scol = slice(sh * 1024, (sh + 1) * 1024)
            kcol = slice(sh * 512, (sh + 1) * 512)
            kh = QKh[64:128, scol].rearrange("p (u two w) -> p u two w",
                                             u=4, two=2)[:, :, 0, :]
            kl = LoQK[64:128, scol].rearrange("p (u two w) -> p u two w",
                                              u=4, two=2)[:, :, 0, :]
            nc.gpsimd.tensor_copy(KrHL[0:64, kcol], kh)
            nc.vector.tensor_copy(KrHL[64:128, kcol], kl)
            nc.gpsimd.tensor_copy(KrH1[0:64, kcol], kh)


        # ---------- phase B: row-max (tail-first) + phase C interleaved ------
        def phase_b(p):
            # tile1: [tail blocks (cu-2, cu-1) | prefix blocks 0..] up to 1024
            # tile2: remaining prefix blocks (cu > 8 only)
            cu = cu_of(p)
            tw = 2 if cu >= 2 else 1
            w1 = min(cu, 8) * P
            lhs = QrHH[:, p * P:(p + 1) * P]
            big = psB.tile([P, 1024], f32, tag="big")
            nc.tensor.matmul(big[:, 0:tw * P], lhs,
                             KrHL[:, (cu - tw) * P:cu * P], start=True, stop=False)
            # fold the causal/parity tail mask in as accumulating matmuls
            if cu == 1:
                nc.tensor.matmul(big[:, 0:P], stairs_sb[:], id16[:],
                                 start=False, stop=True)
            elif p == 31:
                nc.tensor.matmul(big[:, 0:tw * P], nr1[:],
                                 mrhs_sb[:, 256:256 + tw * P],
                                 start=False, stop=True)
            elif p % 2 == 0:
                nc.tensor.matmul(big[:, P:2 * P], stairs_sb[:], id16[:],
                                 start=False, stop=True)
            else:
                nc.tensor.matmul(big[:, 0:tw * P], nr1[:],
                                 mrhs_sb[:, 0:tw * P],
                                 start=False, stop=True)
            pos = tw * P
            while pos < w1:
                w = min(512 - pos % 512 if pos % 512 else 512, w1 - pos)
                nc.tensor.matmul(big[:, pos:pos + w], lhs,
                                 KrHL[:, pos - tw * P:pos - tw * P + w],
                                 start=True, stop=True)
                pos += w
            mxb = work.tile([P, 1], f32, tag="mxb")
            nc.vector.reduce_max(mxb[:], big[:, 0:w1], axis=AX.X)
            npre2 = cu - 8
            if npre2 > 0:
                big2 = psB.tile([P, 1024], f32, tag="big")
                for c0 in range(0, npre2 * P, 512):
                    w = min(512, npre2 * P - c0)
                    nc.tensor.matmul(big2[:, c0:c0 + w], lhs,
                                     KrHL[:, (8 - tw) * P + c0:
                                          (8 - tw) * P + c0 + w],
                                     start=True, stop=True)
                mxa = work.tile([P, 1], f32, tag="mxa")
                nc.vector.reduce_max(mxa[:], big2[:, 0:npre2 * P], axis=AX.X)
                nc.vector.tensor_tensor(m_sb[:, p:p + 1], mxa[:], mxb[:], ALU.max)
            else:
                nc.vector.tensor_copy(m_sb[:, p:p + 1], mxb[:])

        m_dr = dram.tile([32, P], f16)

        def m_group(g, half=None):
            # flatten m for a group of q-blocks -> QrLM row 64
            if half is None:
                p0, np4 = 8 * g, 8
            else:
                p0, np4 = 8 * g + 4 * half, 4
            pc = slice(p0, p0 + np4)
            mc = work.tile([P, 8], f32, tag="mc")
            nc.vector.tensor_scalar_max(mc[:, 0:np4], m_sb[:, pc], NEG16)
            mt_ps = psO.tile([65, 512], f32, tag="o")
            nc.tensor.transpose(mt_ps[0:np4, 0:P], mc[:, 0:np4], id32[:])
            mg16 = work.tile([8, P], f16, tag="mg16")
            nc.vector.tensor_copy(mg16[0:np4, :], mt_ps[0:np4, 0:P])
            mg32 = work.tile([8, P], f32, tag="mg32")
            nc.vector.tensor_copy(mg32[0:np4, :], mg16[0:np4, :])
            nc.scalar.dma_start(m_dr[pc, :], mg16[0:np4, :])
            nc.scalar.dma_start(
                QrLM[64:65, p0 * P:(p0 + np4) * P],
                m_dr[pc, :].rearrange("c w -> (c w)")[None, :])
            nc.scalar.dma_start(m_out[pc, :], mg32[0:np4, :])

        def phase_c(J, on_psA=False):
            cj = 2 * J + 2
            qc = slice(J * 512, (J + 1) * 512)
            order = [cj - 2, cj - 1] + list(range(cj - 2))
            ops_t = psO.tile([65, 512], f32, tag="o")
            if on_psA:
                # per-block tiles from psA so B keeps psB exclusively
                for i, j in enumerate(order):
                    kb = slice(j * P, (j + 1) * P)
                    sp = psA.tile([P, 512], f32, tag="pk")
                    nc.tensor.matmul(sp[:], KrHL[:, kb], QrHH[:, qc],
                                     start=True, stop=False)
                    nc.tensor.matmul(sp[:], KrH1[:, kb], QrLM[:, qc],
                                     start=False, stop=(i >= 2))
                    if i < 2:
                        nc.tensor.matmul(sp[:], id16[:],
                                         stm_sb[:, i * 512:(i + 1) * 512],
                                         start=False, stop=True)
                    a_sb = apool.tile([P, 1024], f16, tag="a")
                    nc.scalar.activation(a_sb[:, 0:512], sp[:], AF.Exp)
                    nc.tensor.matmul(ops_t[:], Vaug[:, j, :], a_sb[:, 0:512],
                                     start=(i == 0), stop=(i == cj - 1))
            else:
                for i0 in range(0, cj, 2):
                    pair = order[i0:i0 + 2]
                    sp = psB.tile([P, 1024], f32, tag="big")
                    for k, j in enumerate(pair):
                        kb = slice(j * P, (j + 1) * P)
                        half = slice(k * 512, (k + 1) * 512)
                        nc.tensor.matmul(sp[:, half], KrHL[:, kb], QrHH[:, qc],
                                         start=True, stop=False)
                        nc.tensor.matmul(sp[:, half], KrH1[:, kb], QrLM[:, qc],
                                         start=False, stop=(i0 > 0))
                        if i0 == 0:
                            nc.tensor.matmul(sp[:, half], id16[:],
                                             stm_sb[:, k * 512:(k + 1) * 512],
                                             start=False, stop=True)
                    a_sb = apool.tile([P, 1024], f16, tag="a")
                    nc.scalar.activation(a_sb[:], sp[:], AF.Exp)
                    for k, j in enumerate(pair):
                        nc.tensor.matmul(ops_t[:], Vaug[:, j, :],
                                         a_sb[:, k * 512:(k + 1) * 512],
                                         start=(i0 == 0 and k == 0),
                                         stop=(i0 + 2 >= cj and k == 1))
            osb0 = work.tile([65, 512], f32, tag="osb0")
            nc.vector.tensor_copy(osb0[:], ops_t[:])
            nc.scalar.dma_start(o_out[:, qc], osb0[:])

        phase_a(0)
        for p in range(6):
            phase_b(p)
        phase_b(6)
        phase_a(1)
        phase_b(7)
        m_group(0)
        phase_b(8)
        phase_b(9)
        phase_a(2)
        for p in range(10, 16):
            phase_b(p)
        m_group(1)
        phase_c(0, on_psA=True)
        phase_c(1, on_psA=True)
        for p in range(16, 22):
            phase_b(p)
        phase_a(3)
        phase_b(22)
        phase_b(23)
        m_group(2)
        phase_c(2, on_psA=True)
        phase_c(3, on_psA=True)
        phase_c(4, on_psA=True)
        phase_c(5, on_psA=True)
        for p in range(24, 32):
            phase_b(p)
        m_group(3)
        phase_c(7, on_psA=True)
        phase_c(6, on_psA=True)

    nc.compile()
    return nc


def _roll_order(h):
    o = np.arange(32).reshape(16, 2)
    return (o if h == 0 else o[:, ::-1]).reshape(-1)


def _host_prep(x, W_Q, W_K, W_V):
    perm = np.empty(64, np.int64)
    perm[:32] = np.arange(32) * 2
    perm[32:] = np.arange(32) * 2 + 1
    wq = (np.asarray(W_Q, np.float64)[:, perm] / 8.0)
    wk = np.asarray(W_K, np.float64)[:, perm]

    def sp(w):
        h = w.astype(np.float16)
        return h, (w - h.astype(np.float64)).astype(np.float16)

    wqh, wql = sp(wq)
    wkh, wkl = sp(wk)
    wvh = np.asarray(W_V, np.float16)
    wpk = np.concatenate([wqh, wkh, wvh, wql, wkl], axis=1)
    wpk = np.ascontiguousarray(wpk.reshape(8, 128, 320).transpose(1, 0, 2))

    pos = np.arange(S, dtype=np.float64)
    inv = 1.0 / (10000.0 ** (2.0 * np.arange(32) / 64.0))
    th = pos[None, :] * inv[:, None]
    c64 = np.concatenate([np.cos(th), np.cos(th)], 0)
    s64 = np.concatenate([-np.sin(th), np.sin(th)], 0)
    cos2 = np.concatenate([c64, c64], 0).astype(np.float32)
    sin2 = np.concatenate([s64, s64], 0).astype(np.float32)
    # pre-swapped sin: sinw[d] = sin2[d^32]
    swapi = np.arange(P) ^ 32
    sinw2 = sin2[swapi]

    di = np.arange(P)[:, None]
    qi = np.arange(P)[None, :]
    stairs = np.where(di > qi, NEG16, 0.0).astype(np.float16)  # [d,q]

    mrhs_h, stm_h, cos_h, sin_h = [], [], [], []
    kin = np.arange(P)[:, None]
    qrel = np.arange(512)[None, :]
    for h in (0, 1):
        mr = np.zeros((1, 512), np.float16)
        od = [0.0, 1.0] if h == 0 else [1.0, 1.0]
        p31 = [0.0, 0.0] if h == 0 else [0.0, 1.0]
        mr[0, 0:128] = od[0]
        mr[0, 128:256] = od[1]
        mr[0, 256:384] = p31[0]
        mr[0, 384:512] = p31[1]
        mrhs_h.append(mr)
        pblk = qrel // P
        gq = np.where(pblk % 2 == 0, pblk + h, pblk - h)
        pats = []
        for wch in (0, 1):
            r = h + 2 * wch
            pats.append(np.where(r * P + kin <= gq * P + qrel % P, 0.0, -65000.0))
        stm_h.append(np.ascontiguousarray(
            np.concatenate(pats, 1).astype(np.float16)))
        order = _roll_order(h)
        colidx = (order[:, None] * P + np.arange(P)[None, :]).reshape(-1)
        cos_h.append(np.ascontiguousarray(cos2[:, colidx]))
        sin_h.append(np.ascontiguousarray(sinw2[:, colidx]))

    from ml_dtypes import float8_e5m2
    wpk8 = np.ascontiguousarray(wpk[:, :, 0:128]).astype(float8_e5m2)
    xts = []
    x64 = np.asarray(x, np.float64)
    for h in (0, 1):
        order = _roll_order(h)
        xr = x64.transpose(0, 2, 1).reshape(B, 8, 128, 32, 128)[:, :, :, order, :]
        xr = xr.reshape(B, 8, 128, S)
        xh = xr.astype(np.float16)
        xl = (xr - xh.astype(np.float64)).astype(float8_e5m2)
        xts.append((np.ascontiguousarray(xh), np.ascontiguousarray(xl)))
    return wpk, wpk8, stairs, mrhs_h, stm_h, cos_h, sin_h, xts


def kernel(x, mask, W_Q, W_K, W_V):
    x = np.asarray(x, np.float32)
    mask = np.asarray(mask)
    if not np.array_equal(mask, np.tril(np.ones((S, S), mask.dtype))):
        Q = x @ W_Q
        K = x @ W_K
        V = x @ W_V
        pos = np.arange(S)
        inv = 1.0 / (10000.0 ** (2.0 * np.arange(32) / 64))
        th = pos[:, None] * inv[None, :]
        sn, cs = np.sin(th), np.cos(th)

        def rp(q):
            x1, x2 = q[..., 0::2], q[..., 1::2]
            o = np.empty_like(q)
            o[..., 0::2] = x1 * cs - x2 * sn
            o[..., 1::2] = x1 * sn + x2 * cs
            return o
        Q, K = rp(Q), rp(K)
        s = np.einsum('bqd,bkd->bqk', Q, K) / 8.0
        s = np.where(mask == 0, -np.inf, s)
        e = np.exp(s - s.max(-1, keepdims=True))
        return (np.einsum('bqk,bkd->bqd', e / e.sum(-1, keepdims=True), V)
                ).astype(np.float32)

    from concourse.bass_utils import run_bass_kernel_spmd
    if "nc" not in _CACHE:
        _CACHE["nc"] = _build_program()
    nc = _CACHE["nc"]

    wpk, wpk8, stairs, mrhs_h, stm_h, cos_h, sin_h, xts = _host_prep(
        x, W_Q, W_K, W_V)
    in_maps = []
    for c in range(8):
        b, h = c // 2, c % 2
        xh, xl = xts[h]
        in_maps.append({
            "xth": xh[b], "xtl": xl[b], "wpk": wpk, "wpk8": wpk8,
            "stairs": stairs, "mrhs": mrhs_h[h], "cosr": cos_h[h],
            "sinw": sin_h[h], "stm": stm_h[h],
        })
    res = run_bass_kernel_spmd(nc, in_maps, core_ids=list(range(8)))

    out = np.empty((B, S, DH), np.float32)
    for b in range(B):
        oa, mm = [], []
        for h in (0, 1):
            rr = res.results[2 * b + h]
            order = _roll_order(h)
            unroll = np.empty(32, np.int64)
            unroll[order] = np.arange(32)     # global block -> rolled position
            o_r = rr["o_out"].astype(np.float64).T.reshape(32, 128, 65)[unroll]
            m_r = rr["m_out"].astype(np.float64)[unroll]
            oa.append(o_r.reshape(S, 65))
            mm.append(m_r.reshape(S))
        M = np.maximum(mm[0], mm[1])
        w = [np.exp(np.minimum(mm[h] - M, 0.0)) for h in (0, 1)]
        num = oa[0][:, :64] * w[0][:, None] + oa[1][:, :64] * w[1][:, None]
        den = oa[0][:, 64] * w[0] + oa[1][:, 64] * w[1]
        out[b] = (num / den[:, None]).astype(np.float32)
    return out
